# revision 5
# baseline (speedup 1.0000x reference)
"""Dynamic per-pixel depthwise 3x3 conv (DYDConv2d) on 8 Trainium2 cores.

Full-tensor contract:
    input : (8, 64, 128, 128) f32
    weight: (8, 64, 3, 3, 128, 128) f32   -- one 3x3 filter per (b, c, oh, ow)
    out   : (8, 64, 128, 128) f32
    out[b,c,oh,ow] = sum_{i,j} xpad[b,c,oh+i,ow+j] * weight[b,c,i,j,oh,ow]
    (stride 1, pad 1)

Sharding: data-parallel over batch B=8 -> one sample per NeuronCore.

fp16 end-to-end on device (harness tolerance is 2e-2; fp16 gives ~1e-3):
halves the dominant ~38 MB/core weight stream to ~19 MB AND doubles DVE
tensor_tensor throughput (2x_1P perf mode needs 16-bit dtype, unit stride,
4B-aligned operands).

Per-core layout: 128 SBUF partitions = (channel c) x (H-half hf), p=c*2+hf.
Each partition holds TWO 66x130 fp16 slabs of its half-image: slab0 is the
zero-padded slab, slab1 the same shifted left one column.  The 3x3 taps are
then free-dim views at EVEN element offsets (kw=0 -> slab0 col0, kw=1 ->
slab1 col0, kw=2 -> slab0 col2), so every DVE operand stays 4B-aligned and
the multiply runs in 2x mode.  Host casts inputs to fp16 and assembles the
slabs; device output is fp16, upcast on host.
"""

import numpy as np

import concourse.bacc as bacc
import concourse.mybir as mybir
from concourse.bass_utils import run_bass_kernel_spmd
from concourse.tile import TileContext

B, C, H, W = 8, 64, 128, 128
KH, KW = 3, 3
HALF = H // 2  # rows per half-image (one partition group)
SLAB_R, SLAB_C = HALF + 2, W + 2  # 66 x 130 padded slab per partition

RT = 16   # output rows per chunk (per half)
GRP = 3   # taps per weight-DMA group

_F16 = mybir.dt.float16

# Number of x slabs shipped per partition.  2 = extra column-shifted copy so
# every tap's column offset is even (4B-aligned -> DVE 2x mode on all 9
# multiplies).  1 = single slab; the three kw=1 taps read at odd offsets and
# drop to 1x, but the x DMA halves.
NSLABS = 2

# tap kw -> (slab index, column offset)
_TAPCOL = {
    2: {0: (0, 0), 1: (1, 0), 2: (0, 2)},
    1: {0: (0, 0), 1: (0, 1), 2: (0, 2)},
}[NSLABS]


def _emit(nc, tc, xs, w, o, rep=1, rt=None):
    rt = RT if rt is None else rt
    wv = w.rearrange("c kh kw (hf r) ww -> c hf (kh kw) r ww", hf=2)
    ov = o.rearrange("c (hf r) ww -> (c hf) r ww", hf=2)

    with tc.tile_pool(name="work", bufs=2) as pool:
        for _r in range(rep):
            xbuf = pool.tile([128, NSLABS, SLAB_R, SLAB_C], _F16, name="xbuf")
            nc.scalar.dma_start(
                out=xbuf[:].rearrange("p s r cc -> p (s r cc)"), in_=xs[:]
            )
            _emit_pass(nc, pool, xbuf, wv, ov, rt)


def _emit_pass(nc, pool, xbuf, wv, ov, rt):
    def xtap(t, r0):
        i, j = divmod(t, KW)
        s, col = _TAPCOL[j]
        return xbuf[:, s, r0 + i : r0 + i + rt, col : col + W]

    for k in range(HALF // rt):
        r0 = k * rt
        acc = pool.tile([128, rt, W], _F16, name="acc")
        tmp = pool.tile([128, rt, W], _F16, name="tmp", bufs=1)
        first = True
        for g0 in range(0, KH * KW, GRP):
            wts = []
            for t in range(g0, min(g0 + GRP, KH * KW)):
                wt = pool.tile([128, rt, W], _F16, name=f"wg{t - g0}")
                nc.sync.dma_start(out=wt[:], in_=wv[:, :, t, r0 : r0 + rt, :])
                wts.append((t, wt))
            for t, wt in wts:
                if first:
                    nc.vector.tensor_tensor(
                        acc[:], xtap(t, r0), wt[:], mybir.AluOpType.mult
                    )
                    first = False
                else:
                    nc.vector.tensor_tensor(
                        tmp[:], xtap(t, r0), wt[:], mybir.AluOpType.mult
                    )
                    nc.vector.tensor_tensor(
                        acc[:], acc[:], tmp[:], mybir.AluOpType.add
                    )
        nc.scalar.dma_start(out=ov[:, r0 : r0 + rt, :], in_=acc[:])


def build_program(rep=1, rt=None, **_ignored):
    nc = bacc.Bacc(
        "TRN2",
        target_bir_lowering=False,
        debug=False,
        enable_asserts=False,
        num_devices=8,
    )
    xs = nc.dram_tensor(
        "xs", [128, NSLABS * SLAB_R * SLAB_C], _F16, kind="ExternalInput"
    ).ap()
    w = nc.dram_tensor("w", [C, KH, KW, H, W], _F16, kind="ExternalInput").ap()
    o = nc.dram_tensor("o", [C, H, W], _F16, kind="ExternalOutput").ap()
    with TileContext(nc) as tc:
        _emit(nc, tc, xs, w, o, rep=rep, rt=rt)
    nc.compile()
    return nc


def make_slab(x_one):
    """Host-side double slab for one sample: [64,128,128] f32 -> [128, 2*66*130] fp16.

    Partition p = c*2 + hf holds rows hf*64-1 .. hf*64+64 of channel c
    (zero-padded at the image border) in a 66x130 col-padded layout;
    slab s=0 unshifted, s=1 shifted left one column (for 4B-aligned kw=1
    taps).
    """
    slab = np.zeros((C, 2, NSLABS, SLAB_R, SLAB_C), dtype=np.float16)  # (c, hf, s, r, col)
    x16 = x_one.astype(np.float16)
    # half 0: slab rows 1..65 <- x rows 0..64 (row 0 stays zero: top pad)
    slab[:, 0, 0, 1 : HALF + 2, 1 : W + 1] = x16[:, 0 : HALF + 1, :]
    # half 1: slab rows 0..64 <- x rows 63..127 (row 65 stays zero: bottom pad)
    slab[:, 1, 0, 0 : HALF + 1, 1 : W + 1] = x16[:, HALF - 1 : H, :]
    if NSLABS == 2:
        slab[:, :, 1, :, 0 : SLAB_C - 1] = slab[:, :, 0, :, 1:SLAB_C]
    return slab.reshape(128, NSLABS * SLAB_R * SLAB_C)


def make_w(w_one):
    """Host-side fp16 cast of one sample's weights: [64,3,3,128,128]."""
    return np.ascontiguousarray(w_one.astype(np.float16))


_CACHE = {}


def _spot_check(out, input, weight, n=16):
    """Max rel err of `out` vs host reference on n random output rows.

    Cheap (n*9 row FMAs on host) guard against rare transient device
    faults; fp16 end-to-end lands ~2e-4 here, garbage lands ~1.
    """
    rng = np.random.default_rng(0)
    xpad = np.pad(input, ((0, 0), (0, 0), (1, 1), (1, 1)))
    worst = 0.0
    for b, c, r in zip(
        rng.integers(0, B, n), rng.integers(0, C, n), rng.integers(0, H, n)
    ):
        exp = np.zeros(W, np.float32)
        for i in range(KH):
            for j in range(KW):
                exp += xpad[b, c, r + i, j : j + W] * weight[b, c, i, j, r, :]
        scale = max(float(np.abs(exp).max()), 1.0)
        worst = max(worst, float(np.abs(out[b, c, r] - exp).max()) / scale)
    return worst


def kernel(input, weight, _trace=False):
    input = np.asarray(input, dtype=np.float32)
    weight = np.asarray(weight, dtype=np.float32)
    assert input.shape == (B, C, H, W), input.shape
    assert weight.shape == (B, C, KH, KW, H, W), weight.shape

    if "nc" not in _CACHE:
        _CACHE["nc"] = build_program()
    nc = _CACHE["nc"]

    in_maps = [
        {"xs": make_slab(input[b]), "w": make_w(weight[b])} for b in range(B)
    ]
    for attempt in range(3):
        res = run_bass_kernel_spmd(
            nc, in_maps, core_ids=list(range(B)), trace=_trace
        )
        _CACHE["last_result"] = res
        out = np.stack([res.results[b]["o"] for b in range(B)], axis=0)
        out = out.astype(np.float32)
        if _spot_check(out, input, weight) < 8e-3:
            break
    return out
